# revision 1
# baseline (speedup 1.0000x reference)
"""CornerPool module kernel for Trainium2 (Bass/Tile), 8-core batch-parallel.

Model (per sample, C=256, H=W=128):
  t = relu(bn(conv3x3(x, w_t)));  tp = reverse-cummax_H(t)
  l = relu(bn(conv3x3(x, w_l)));  lp = reverse-cummax_W(l)
  b = relu(bn(conv3x3(x, w_b)));  bp = cummax_H(b)
  r = relu(bn(conv3x3(x, w_r)));  rp = cummax_W(r)
  tl = relu(bn3(conv3x3(tp+lp)) + bn1(conv1x1(x)));  out_tl = relu(bn(conv3x3(tl)))
  br = relu(bn3(conv3x3(bp+rp)) + bn1(conv1x1(x)));  out_br = relu(bn(conv3x3(br)))

Strategy: one sample per NeuronCore (B=8). All convs lowered to f32r
(full-rate fp32) matmuls over 128-channel tiles with N=512 (4 image rows)
PSUM accumulation groups; BN scale folded into weights on host, bias applied
in the ScalarE relu epilogue. Corner pools: H-direction via 2-step
shifted-max doubling + inter-strip carry, W-direction via the native DVE
prefix-scan instruction (per image row, reversed AP for left-pool).
Intermediates (pooled maps, tp+lp sums, tl/br) round-trip through padded
internal-DRAM scratch so every 3x3 conv reads zero-padded halos uniformly.
"""

import numpy as np

_P = 128          # partitions / channel tile
_SR = 4           # image rows per strip (N = _SR*128 = 512)


def _prep_host(inputs):
    """Fold BN scales into weights, build lhsT-layout weight arrays and the
    combined bias table. Returns dict of shared input arrays."""
    f32 = np.float32

    def scaled(name):
        w = np.asarray(inputs["w_" + name], f32)
        s = np.asarray(inputs["s_" + name], f32)
        return w * s[:, None, None, None]

    def bias(name):
        return np.asarray(inputs["b_" + name], f32)

    # stage A convs: [128co, 256ci, 3, 3] -> [128k, 18(ci_t*9+dydx), 128m]
    def layA(w):
        a = w.transpose(1, 2, 3, 0).reshape(2, 128, 9, 128)   # ci_t,k,dydx,m
        return np.ascontiguousarray(a.transpose(1, 0, 2, 3).reshape(128, 18, 128))

    wa = np.stack([layA(scaled(n)) for n in ("t", "l", "b", "r")])  # [4,128,18,128]

    # stage C: w3 [256co,128ci,3,3] -> [k, co_t*9+dydx, m];
    #          w1 [256co,256ci,1,1] -> [k, co_t*2+ci_t, m]; concat -> 22 slices
    def layC(w3, w1):
        a3 = w3.transpose(1, 2, 3, 0).reshape(128, 9, 2, 128)     # k,dydx,co_t,m
        a3 = a3.transpose(0, 2, 1, 3).reshape(128, 18, 128)
        a1 = w1[:, :, 0, 0].T.reshape(2, 128, 2, 128)             # ci_t,k,co_t,m
        a1 = a1.transpose(1, 2, 0, 3).reshape(128, 4, 128)        # k, co_t*2+ci_t, m
        return np.ascontiguousarray(np.concatenate([a3, a1], axis=1))

    wc = np.stack([layC(scaled("tl3"), scaled("tl1")),
                   layC(scaled("br3"), scaled("br1"))])            # [2,128,22,128]

    # stage D: [256co,256ci,3,3] -> [k, co_t, ci_t*9+dydx, m]
    def layD(w):
        a = w.transpose(1, 2, 3, 0).reshape(2, 128, 3, 3, 2, 128)  # ci_t,k,dy,dx,co_t,m
        a = a.transpose(1, 4, 0, 2, 3, 5).reshape(128, 2, 18, 128)
        return np.ascontiguousarray(a)

    wd = np.stack([layD(scaled("tlo")), layD(scaled("bro"))])      # [2,128,2,18,128]

    bias_rows = [bias("t"), bias("l"), bias("b"), bias("r")]       # 0..3
    for bi, (n3, n1) in enumerate((("tl3", "tl1"), ("br3", "br1"))):
        comb = bias(n3) + bias(n1)                                 # [256]
        bias_rows += [comb[:128], comb[128:]]                      # 4+bi*2+co_t
    for n in ("tlo", "bro"):
        bb = bias(n)
        bias_rows += [bb[:128], bb[128:]]                          # 8+bi*2+co_t
    bias_all = np.ascontiguousarray(np.stack(bias_rows).T).astype(f32)  # [128,12]

    return {"wa": wa, "wc": wc, "wd": wd, "bias": bias_all}


def _pad_x_sample(xs, H):
    """[256,H,128] f32 -> [2,128,H+2,130] zero-padded."""
    xp = np.zeros((2, 128, H + 2, 130), np.float32)
    xp[:, :, 1:H + 1, 1:129] = xs.reshape(2, 128, H, 128)
    return xp


def _build(H):
    """Build the Bass module for one core (one sample of height H)."""
    import concourse.bacc as bacc
    import concourse.mybir as mybir
    import concourse.tile as tile

    dt = mybir.dt
    Alu = mybir.AluOpType
    Act = mybir.ActivationFunctionType
    S = H // _SR
    HP = H + 2
    NPIX = HP * 130

    nc = bacc.Bacc("TRN2", target_bir_lowering=False, debug=False)

    xpad = nc.dram_tensor("xpad", [2, 128, HP, 130], dt.float32, kind="ExternalInput")
    wa_d = nc.dram_tensor("wa", [4, 128, 18, 128], dt.float32, kind="ExternalInput")
    wc_d = nc.dram_tensor("wc", [2, 128, 22, 128], dt.float32, kind="ExternalInput")
    wd_d = nc.dram_tensor("wd", [2, 128, 2, 18, 128], dt.float32, kind="ExternalInput")
    bias_d = nc.dram_tensor("bias", [128, 12], dt.float32, kind="ExternalInput")
    out_tl = nc.dram_tensor("out_tl", [256, H, 128], dt.float32, kind="ExternalOutput")
    out_br = nc.dram_tensor("out_br", [256, H, 128], dt.float32, kind="ExternalOutput")

    # internal DRAM scratch (f32r, produced rounded on-chip)
    tp_d = nc.dram_tensor("tp_s", [128, H, 128], dt.float32r)
    bp_d = nc.dram_tensor("bp_s", [128, H, 128], dt.float32r)
    sum_d = nc.dram_tensor("sum_s", [2, 128, HP, 130], dt.float32r)
    tlb_d = nc.dram_tensor("tlb_s", [2, 2, 128, HP, 130], dt.float32r)

    with tile.TileContext(nc) as tc:
        import contextlib
        with contextlib.ExitStack() as ctx:
            xpool = ctx.enter_context(tc.tile_pool(name="xp", bufs=1))
            wpool = ctx.enter_context(tc.tile_pool(name="wp", bufs=3))
            spool = ctx.enter_context(tc.tile_pool(name="sp", bufs=2))
            wpool2 = ctx.enter_context(tc.tile_pool(name="wide", bufs=3))
            hpool = ctx.enter_context(tc.tile_pool(name="hp", bufs=3))
            cpool = ctx.enter_context(tc.tile_pool(name="cp", bufs=2))
            mpool = ctx.enter_context(tc.tile_pool(name="mp", bufs=1))
            pspool = ctx.enter_context(tc.tile_pool(name="ps", bufs=8, space="PSUM"))

            # ---- preamble: x, biases, zero borders --------------------
            nch = 4
            bounds = [HP - (HP * k) // nch for k in range(nch + 1)]  # desc
            xt0 = xpool.tile([128, NPIX], dt.float32r, tag="x0")
            xt1 = xpool.tile([128, NPIX], dt.float32r, tag="x1")
            xt = [xt0, xt1]

            def load_x_chunk(k):
                for ci, eng in ((0, nc.sync), (1, nc.scalar)):
                    a, b = bounds[k + 1], bounds[k]
                    seg = xt[ci][:, a * 130:b * 130]
                    eng.dma_start(seg,
                                  xpad.ap()[ci][:, a:b, :].bitcast(dt.float32r))
                    nc.vector.tensor_copy(seg, seg.bitcast(dt.float32))

            load_x_chunk(0)
            xr = [t[:].rearrange("p (a b) -> p a b", b=130) for t in xt]

            bt = mpool.tile([128, 12], dt.float32, tag="bias")
            nc.sync.dma_start(bt[:], bias_d.ap())


            def load_w(src_ap, nsl):
                t = wpool.tile([128, nsl, 128], dt.float32r, tag="w")
                h = nsl // 2
                r = src_ap.bitcast(dt.float32r)
                nc.sync.dma_start(t[:, :h], r[:, :h])
                nc.scalar.dma_start(t[:, h:], r[:, h:])
                nc.vector.tensor_copy(t[:], t[:].bitcast(dt.float32))
                return t

            def conv_a_mms(ps, w, s):
                i = 0
                for ci in range(2):
                    for dy in range(3):
                        for dx in range(3):
                            nc.tensor.matmul(
                                ps[:], w[:, ci * 9 + dy * 3 + dx],
                                xr[ci][:, _SR * s + dy:_SR * s + dy + _SR,
                                       dx:dx + 128],
                                start=(i == 0), stop=(i == 17))
                            i += 1

            def act_strip(ps, brow, dtype=dt.float32r):
                t = spool.tile([128, _SR, 128], dtype, tag="ct")
                nc.scalar.activation(t[:].rearrange("p a b -> p (a b)"), ps[:],
                                     Act.Relu, bias=bt[:, brow:brow + 1],
                                     scale=1.0)
                return t

            def act_strip_wide(ps, brow):
                # [128, 4, 130] with zeroed w-border columns; ACT fills interior
                t = wpool2.tile([128, _SR, 130], dt.float32r, tag="cw")
                nc.gpsimd.memset(t[:, :, 0:1].bitcast(dt.float32), 0.0)
                nc.gpsimd.memset(t[:, :, 129:130].bitcast(dt.float32), 0.0)
                nc.scalar.activation(t[:, :, 1:129], ps[:],
                                     Act.Relu, bias=bt[:, brow:brow + 1],
                                     scale=1.0)
                return t

            # ---- pass T: conv t, reverse cummax over H (strips desc) --
            w_t = load_w(wa_d.ap()[0], 18)
            for _k in range(1, nch):
                load_x_chunk(_k)
            zt = mpool.tile([128, 130], dt.float32r, tag="zero")
            nc.vector.memset(zt[:].bitcast(dt.float32), 0.0)
            for i, buf in enumerate((sum_d.ap()[0], sum_d.ap()[1],
                                     tlb_d.ap()[0, 0], tlb_d.ap()[0, 1],
                                     tlb_d.ap()[1, 0], tlb_d.ap()[1, 1])):
                eng = nc.sync if i % 2 else nc.scalar
                eng.dma_start(buf[:, 0, :], zt[:, :130])
                eng.dma_start(buf[:, HP - 1, :], zt[:, :130])

            carry = cpool.tile([128, 1, 128], dt.float32r, tag="cryT")
            nc.vector.memset(carry[:].bitcast(dt.float32), 0.0)
            for s in reversed(range(S)):
                ps = pspool.tile([128, 512], dt.float32, tag="ps")
                conv_a_mms(ps, w_t, s)
                ct = act_strip(ps, 0)
                nc.vector.tensor_tensor(ct[:, 0:3], ct[:, 0:3], ct[:, 1:4], Alu.max)
                nc.vector.tensor_tensor(ct[:, 0:2], ct[:, 0:2], ct[:, 2:4], Alu.max)
                nc.vector.tensor_tensor(ct[:], ct[:],
                                        carry[:].broadcast_to([128, _SR, 128]),
                                        Alu.max)
                if s != 0:
                    nxt = cpool.tile([128, 1, 128], dt.float32r, tag="cryT")
                    nc.vector.tensor_copy(nxt[:], ct[:, 0:1])
                    carry = nxt
                nc.sync.dma_start(tp_d.ap()[:, _SR * s:_SR * (s + 1), :], ct[:])

            # ---- pass B: conv b, forward cummax over H (asc) ----------
            w_b = load_w(wa_d.ap()[2], 18)
            carry = cpool.tile([128, 1, 128], dt.float32r, tag="cryB")
            nc.vector.memset(carry[:].bitcast(dt.float32), 0.0)
            for s in range(S):
                ps = pspool.tile([128, 512], dt.float32, tag="ps")
                conv_a_mms(ps, w_b, s)
                ct = act_strip(ps, 2)
                p1 = spool.tile([128, _SR, 128], dt.float32r, tag="p1")
                nc.vector.tensor_tensor(p1[:, 1:4], ct[:, 1:4], ct[:, 0:3], Alu.max)
                nc.vector.tensor_copy(p1[:, 0:1], ct[:, 0:1])
                nc.vector.tensor_tensor(p1[:, 2:4], p1[:, 2:4], p1[:, 0:2], Alu.max)
                nc.vector.tensor_tensor(p1[:], p1[:],
                                        carry[:].broadcast_to([128, _SR, 128]),
                                        Alu.max)
                if s != S - 1:
                    nxt = cpool.tile([128, 1, 128], dt.float32r, tag="cryB")
                    nc.vector.tensor_copy(nxt[:], p1[:, 3:4])
                    carry = nxt
                nc.sync.dma_start(bp_d.ap()[:, _SR * s:_SR * (s + 1), :], p1[:])

            # ---- pass L: conv l, reverse cummax over W, add tp --------
            w_l = load_w(wa_d.ap()[1], 18)
            for s in range(S):
                ps = pspool.tile([128, 512], dt.float32, tag="ps")
                conv_a_mms(ps, w_l, s)
                ct = act_strip_wide(ps, 1)
                for h in range(_SR):
                    v = ct[:, h, 1:129][:, ::-1]
                    nc.vector.tensor_tensor_scan(v, v, v, 0.0,
                                                 op0=Alu.max, op1=Alu.bypass)
                tps = spool.tile([128, _SR, 128], dt.float32r, tag="tps")
                nc.sync.dma_start(tps[:], tp_d.ap()[:, _SR * s:_SR * (s + 1), :])
                nc.vector.tensor_tensor(ct[:, :, 1:129], ct[:, :, 1:129],
                                        tps[:], Alu.add)
                nc.sync.dma_start(
                    sum_d.ap()[0][:, 1 + _SR * s:1 + _SR * (s + 1), :], ct[:])

            # ---- pass R: conv r, forward cummax over W, add bp --------
            w_r = load_w(wa_d.ap()[3], 18)
            for s in range(S):
                ps = pspool.tile([128, 512], dt.float32, tag="ps")
                conv_a_mms(ps, w_r, s)
                ct = act_strip_wide(ps, 3)
                for h in range(_SR):
                    v = ct[:, h, 1:129]
                    nc.vector.tensor_tensor_scan(v, v, v, 0.0,
                                                 op0=Alu.max, op1=Alu.bypass)
                tps = spool.tile([128, _SR, 128], dt.float32r, tag="tps")
                nc.sync.dma_start(tps[:], bp_d.ap()[:, _SR * s:_SR * (s + 1), :])
                nc.vector.tensor_tensor(ct[:, :, 1:129], ct[:, :, 1:129],
                                        tps[:], Alu.add)
                nc.sync.dma_start(
                    sum_d.ap()[1][:, 1 + _SR * s:1 + _SR * (s + 1), :], ct[:])

            # ---- stage C: tl = relu(conv3x3(sum) + conv1x1(x)) --------
            for bi in range(2):
                w_c = load_w(wc_d.ap()[bi], 22)
                for s in range(S):
                    sums = hpool.tile([128, 6, 130], dt.float32r, tag="sums")
                    nc.sync.dma_start(sums[:],
                                      sum_d.ap()[bi][:, _SR * s:_SR * s + 6, :])
                    for co in range(2):
                        ps = pspool.tile([128, 512], dt.float32, tag="ps")
                        i = 0
                        for dy in range(3):
                            for dx in range(3):
                                nc.tensor.matmul(
                                    ps[:], w_c[:, co * 9 + dy * 3 + dx],
                                    sums[:, dy:dy + _SR, dx:dx + 128],
                                    start=(i == 0), stop=False)
                                i += 1
                        for ci in range(2):
                            nc.tensor.matmul(
                                ps[:], w_c[:, 18 + co * 2 + ci],
                                xr[ci][:, 1 + _SR * s:1 + _SR * (s + 1), 1:129],
                                start=False, stop=(ci == 1))
                        cst = act_strip_wide(ps, 4 + bi * 2 + co)
                        nc.sync.dma_start(
                            tlb_d.ap()[bi, co][:, 1 + _SR * s:1 + _SR * (s + 1),
                                               :], cst[:])

            # ---- stage D: out = relu(conv3x3(tl)) ---------------------
            for bi in range(2):
                wd0 = load_w(wd_d.ap()[bi, :, 0], 18)
                wd1 = load_w(wd_d.ap()[bi, :, 1], 18)
                out_d = out_tl if bi == 0 else out_br
                for s in range(S):
                    din = []
                    for ci in range(2):
                        t = hpool.tile([128, 6, 130], dt.float32r, tag="dls")
                        nc.sync.dma_start(
                            t[:], tlb_d.ap()[bi, ci][:, _SR * s:_SR * s + 6, :])
                        din.append(t)
                    for co, w in ((0, wd0), (1, wd1)):
                        ps = pspool.tile([128, 512], dt.float32, tag="ps")
                        i = 0
                        for ci in range(2):
                            for dy in range(3):
                                for dx in range(3):
                                    nc.tensor.matmul(
                                        ps[:], w[:, ci * 9 + dy * 3 + dx],
                                        din[ci][:, dy:dy + _SR, dx:dx + 128],
                                        start=(i == 0), stop=(i == 17))
                                    i += 1
                        ot = act_strip(ps, 8 + bi * 2 + co, dtype=dt.float32)
                        nc.sync.dma_start(
                            out_d.ap()[co * 128:(co + 1) * 128,
                                       _SR * s:_SR * (s + 1), :], ot[:])

    nc.compile()
    return nc


_NC_CACHE = {}


def _get_nc(H):
    if H not in _NC_CACHE:
        _NC_CACHE[H] = _build(H)
    return _NC_CACHE[H]


def kernel(**inputs):
    from concourse import bass_utils

    x = np.asarray(inputs["x"], np.float32)
    B, C, H, W = x.shape
    assert (C, W) == (256, 128) and H % _SR == 0

    shared = _prep_host(inputs)
    nc = _get_nc(H)

    in_maps = []
    for b in range(B):
        m = dict(shared)
        m["xpad"] = _pad_x_sample(x[b], H)
        in_maps.append(m)

    import os
    trace = bool(int(os.environ.get("KERNEL_TRACE", "0")))
    res = bass_utils.run_bass_kernel_spmd(
        nc, in_maps, core_ids=list(range(B)), trace=trace)
    kernel.last_result = res

    otl = np.stack([res.results[b]["out_tl"].reshape(256, H, 128)
                    for b in range(B)])
    obr = np.stack([res.results[b]["out_br"].reshape(256, H, 128)
                    for b in range(B)])
    return otl, obr



# revision 6
# speedup vs baseline: 1.1792x; 1.1792x over previous
"""CornerPool module kernel for Trainium2 (Bass/Tile), 8-core batch-parallel.

Model (per sample, C=256, H=W=128):
  t = relu(bn(conv3x3(x, w_t)));  tp = reverse-cummax_H(t)
  l = relu(bn(conv3x3(x, w_l)));  lp = reverse-cummax_W(l)
  b = relu(bn(conv3x3(x, w_b)));  bp = cummax_H(b)
  r = relu(bn(conv3x3(x, w_r)));  rp = cummax_W(r)
  tl = relu(bn3(conv3x3(tp+lp)) + bn1(conv1x1(x)));  out_tl = relu(bn(conv3x3(tl)))
  br = relu(bn3(conv3x3(bp+rp)) + bn1(conv1x1(x)));  out_br = relu(bn(conv3x3(br)))

Strategy: one sample per NeuronCore (B=8). All 3x3 convs use 1D Winograd
F(2,3) along W: per output-column pair, 4 transformed input streams
(d0-d2, d1+d2, d2-d1, d1-d3; computed on GpSimd) are contracted against
G-transformed weights (folded on host) in 4 PSUM accumulation groups of
f32r matmuls (N=512 = 8 image rows x 64 column pairs), then recombined
(y0 = m0+m1+m2, y1 = m1-m2-m3) on the Vector engine. This cuts PE matmul
rows to 2/3 of direct conv. The 1x1 convs stay direct. BN scale is folded
into weights, bias applied in the ScalarE relu epilogue.

Pooling is restructured so all four stage-A convs share one forward strip
pass (reusing the transformed x): W-direction pools use the DVE prefix-scan
per row; the bottom (forward-H) pool folds in-pass via a shifted-max ladder
with an inter-strip carry; the top (reverse-H) pool runs as a short reverse
pass over the stored t map. Intermediate maps round-trip DRAM in bf16.
"""

import numpy as np

_P = 128
_SR = 8            # image rows per strip


def _prep_host(inputs):
    """Fold BN scales into weights, apply the F(2,3) weight transform along
    W (G = [[1,0,0],[.5,.5,.5],[.5,-.5,.5],[0,0,1]]), build lhsT-layout
    arrays and the combined bias table."""
    f32 = np.float32

    def scaled(name):
        w = np.asarray(inputs["w_" + name], f32)
        s = np.asarray(inputs["s_" + name], f32)
        return (w * s[:, None, None, None]).astype(np.float64)

    def bias(name):
        return np.asarray(inputs["b_" + name], f32)

    def gtrans(w):
        # w [co, ci, 3dy, 3dx] -> g [co, ci, 3dy, 4k] along dx
        w0, w1, w2 = w[..., 0], w[..., 1], w[..., 2]
        return np.stack([w0, (w0 + w1 + w2) * 0.5, (w0 - w1 + w2) * 0.5, w2],
                        axis=-1)

    # stage A: [128co, 256ci, 3, 3] -> [128k, ci_t*12 + dy*4 + k, 128co]
    def layA(w):
        g = gtrans(w)                                   # [128,256,3,4]
        a = g.transpose(1, 2, 3, 0)                     # [256ci,3dy,4k,128co]
        a = a.reshape(2, 128, 3, 4, 128)                # ci_t,kpart,dy,k,co
        a = a.transpose(1, 0, 2, 3, 4).reshape(128, 24, 128)
        return np.ascontiguousarray(a.astype(f32))

    wa = np.stack([layA(scaled(n)) for n in ("t", "l", "b", "r")])  # [4,128,24,128]

    # stage C: 3x3 [256co,128ci,3,3] -> [128, co_t*12 + dy*4 + k, 128]
    #          1x1 [256co,256ci,1,1] -> [128, 24 + co_t*2 + ci_t, 128]
    def layC(w3, w1s):
        g = gtrans(w3)                                  # [256,128,3,4]
        a3 = g.transpose(1, 2, 3, 0).reshape(128, 3, 4, 2, 128)
        a3 = a3.transpose(0, 3, 1, 2, 4).reshape(128, 24, 128)
        a1 = w1s[:, :, 0, 0].T.reshape(2, 128, 2, 128).astype(np.float64)
        a1 = a1.transpose(1, 2, 0, 3).reshape(128, 4, 128)
        return np.ascontiguousarray(
            np.concatenate([a3, a1], axis=1).astype(f32))

    def sc1(name):
        w = np.asarray(inputs["w_" + name], f32)
        s = np.asarray(inputs["s_" + name], f32)
        return w * s[:, None, None, None]

    wc = np.stack([layC(scaled("tl3"), sc1("tl1")),
                   layC(scaled("br3"), sc1("br1"))])     # [2,128,28,128]

    # stage D: [256co,256ci,3,3] -> [128, co_t, ci_t*12 + dy*4 + k, 128]
    def layD(w):
        g = gtrans(w)                                   # [256,256,3,4]
        a = g.transpose(1, 2, 3, 0)                     # [256ci,3,4,256co]
        a = a.reshape(2, 128, 3, 4, 2, 128)             # ci_t,k,dy,kk,co_t,co
        a = a.transpose(1, 4, 0, 2, 3, 5).reshape(128, 2, 24, 128)
        return np.ascontiguousarray(a.astype(f32))

    wd = np.stack([layD(scaled("tlo")), layD(scaled("bro"))])  # [2,128,2,24,128]

    bias_rows = [bias("t"), bias("l"), bias("b"), bias("r")]
    for n3, n1 in (("tl3", "tl1"), ("br3", "br1")):
        comb = bias(n3) + bias(n1)
        bias_rows += [comb[:128], comb[128:]]
    for n in ("tlo", "bro"):
        bb = bias(n)
        bias_rows += [bb[:128], bb[128:]]
    bias_all = np.ascontiguousarray(np.stack(bias_rows).T).astype(f32)  # [128,12]

    return {"wa": wa, "wc": wc, "wd": wd, "bias": bias_all}


def _pad_x_sample(xs, H):
    """[256,H,128] f32 -> [2,128,H+2,130] zero-padded."""
    xp = np.zeros((2, 128, H + 2, 130), np.float32)
    xp[:, :, 1:H + 1, 1:129] = xs.reshape(2, 128, H, 128)
    return xp


def _build(H):
    """Build the Bass module for one core (one sample of height H)."""
    import concourse.bacc as bacc
    import concourse.mybir as mybir
    import concourse.tile as tile

    dt = mybir.dt
    Alu = mybir.AluOpType
    Act = mybir.ActivationFunctionType
    S = H // _SR          # strips
    HP = H + 2
    NR = _SR + 2          # rows per strip incl. halo

    nc = bacc.Bacc("TRN2", target_bir_lowering=False, debug=False)

    xpad = nc.dram_tensor("xpad", [2, 128, HP, 130], dt.float32, kind="ExternalInput")
    wa_d = nc.dram_tensor("wa", [4, 128, 24, 128], dt.float32, kind="ExternalInput")
    wc_d = nc.dram_tensor("wc", [2, 128, 28, 128], dt.float32, kind="ExternalInput")
    wd_d = nc.dram_tensor("wd", [2, 128, 2, 24, 128], dt.float32, kind="ExternalInput")
    bias_d = nc.dram_tensor("bias", [128, 12], dt.float32, kind="ExternalInput")
    out_tl = nc.dram_tensor("out_tl", [256, H, 128], dt.float32, kind="ExternalOutput")
    out_br = nc.dram_tensor("out_br", [256, H, 128], dt.float32, kind="ExternalOutput")

    # internal DRAM scratch (bf16)
    t_d = nc.dram_tensor("t_s", [128, H, 128], dt.bfloat16)
    lp_d = nc.dram_tensor("lp_s", [128, H, 128], dt.bfloat16)
    sum_d = nc.dram_tensor("sum_s", [2, 128, HP, 130], dt.bfloat16)
    tlb_d = nc.dram_tensor("tlb_s", [2, 2, 128, HP, 130], dt.bfloat16)

    with tile.TileContext(nc) as tc:
        import contextlib
        with contextlib.ExitStack() as ctx:
            mpool = ctx.enter_context(tc.tile_pool(name="mp", bufs=1))
            cpool = ctx.enter_context(tc.tile_pool(name="cp", bufs=2))
            pspool = ctx.enter_context(tc.tile_pool(name="ps", bufs=8, space="PSUM"))

            bt = mpool.tile([128, 12], dt.float32, tag="bias")
            nc.scalar.dma_start(bt[:], bias_d.ap())

            # zero border rows of padded scratch maps
            ztb = mpool.tile([128, 130], dt.bfloat16, tag="zerob")
            nc.vector.memset(ztb[:], 0.0)
            for buf in (sum_d.ap()[0], sum_d.ap()[1],
                        tlb_d.ap()[0, 0], tlb_d.ap()[0, 1],
                        tlb_d.ap()[1, 0], tlb_d.ap()[1, 1]):
                nc.sync.dma_start(buf[:, 0, :], ztb[:])
                nc.sync.dma_start(buf[:, HP - 1, :], ztb[:])

            def load_w(pool, src_ap, nsl, tag, eng=None):
                eng = eng or nc.scalar
                t = pool.tile([128, nsl, 128], dt.float32r, tag=tag)
                h = nsl // 2
                r = src_ap.bitcast(dt.float32r)
                eng.dma_start(t[:, :h], r[:, :h])
                eng.dma_start(t[:, h:], r[:, h:])
                nc.vector.tensor_copy(t[:], t[:].bitcast(dt.float32))
                return t

            # F(2,3) input transform: xs [128, NR, 130] -> dx [128, 4, NR, 64]
            def wtransform(pool, xs, dtag):
                xv = xs.rearrange("p r (g t) -> p r g t", t=2)
                d0 = xv[:, :, 0:64, 0]
                d1 = xv[:, :, 0:64, 1]
                d2 = xv[:, :, 1:65, 0]
                d3 = xv[:, :, 1:65, 1]
                dx = pool.tile([128, 4, NR, 64], dt.float32r, tag=dtag)
                nc.gpsimd.tensor_tensor(dx[:, 0], d0, d2, Alu.subtract)
                nc.gpsimd.tensor_tensor(dx[:, 1], d1, d2, Alu.add)
                nc.gpsimd.tensor_tensor(dx[:, 2], d2, d1, Alu.subtract)
                nc.gpsimd.tensor_tensor(dx[:, 3], d1, d3, Alu.subtract)
                return dx

            # 4 m-group matmuls + recombine -> Y [128, SR, 64, 2] f32r
            def wino_mm(ypool, wt, wof, dxs):
                ms = []
                for k in range(4):
                    ps = pspool.tile([128, 512], dt.float32, tag="ps")
                    n = len(dxs) * 3
                    i = 0
                    for ci, dx in enumerate(dxs):
                        for dy in range(3):
                            nc.tensor.matmul(
                                ps[:], wt[:, wof + ci * 12 + dy * 4 + k],
                                dx[:, k, dy:dy + _SR, :],
                                start=(i == 0), stop=(i == n - 1))
                            i += 1
                    ms.append(ps[:].rearrange("p (r g) -> p r g", g=64))
                y = ypool.tile([128, _SR, 64, 2], dt.float32r, tag="y", bufs=3)
                tc_ = ypool.tile([128, _SR, 64], dt.float32r, tag="yc", bufs=3)
                ta = ypool.tile([128, _SR, 64], dt.float32r, tag="ya", bufs=3)
                nc.vector.tensor_copy(tc_[:], ms[1])
                nc.vector.tensor_tensor(ta[:], tc_[:], ms[2], Alu.add)
                nc.vector.tensor_tensor(y[:, :, :, 0], ta[:], ms[0], Alu.add)
                nc.vector.tensor_tensor(ta[:], tc_[:], ms[2], Alu.subtract)
                nc.vector.tensor_tensor(y[:, :, :, 1], ta[:], ms[3], Alu.subtract)
                return y

            def act_to(pool, y, brow, tag, dtype=dt.bfloat16, bufs=2):
                t = pool.tile([128, _SR, 128], dtype, tag=tag, bufs=bufs)
                nc.scalar.activation(t[:], y[:],
                                     Act.Relu, bias=bt[:, brow:brow + 1],
                                     scale=1.0)
                return t

            # ---- pass A: 4 convs, W-pools, forward-H pool ------------------
            with contextlib.ExitStack() as actx:
                wpool = actx.enter_context(tc.tile_pool(name="wpA", bufs=1))
                xpool = actx.enter_context(tc.tile_pool(name="xpA", bufs=2))
                dpool = actx.enter_context(tc.tile_pool(name="dpA", bufs=2))
                ypool = actx.enter_context(tc.tile_pool(name="ypA", bufs=3))
                apool = actx.enter_context(tc.tile_pool(name="apA", bufs=2))

                w_a = [load_w(wpool, wa_d.ap()[0], 24, "wa0")]
                carry_b = cpool.tile([128, 1, 128], dt.bfloat16, tag="cryB")
                nc.vector.memset(carry_b[:], 0.0)

                for s in range(S):
                    xs = []
                    for ci in range(2):
                        t = xpool.tile([128, NR, 130], dt.float32, tag=f"xs{ci}")
                        nc.scalar.dma_start(
                            t[:], xpad.ap()[ci][:, _SR * s:_SR * s + NR, :])
                        xs.append(t)
                    dxs = [wtransform(dpool, xs[ci], f"dx{ci}") for ci in range(2)]
                    if s == 0:
                        for i in range(1, 4):
                            w_a.append(load_w(wpool, wa_d.ap()[i], 24, f"wa{i}",
                                              eng=nc.sync))

                    # conv T: store raw t map (bf16)
                    y = wino_mm(ypool, w_a[0], 0, dxs)
                    tb = act_to(apool, y, 0, "aT")
                    nc.sync.dma_start(t_d.ap()[:, _SR * s:_SR * (s + 1), :], tb[:])

                    # conv L: reverse cummax along W, store lp
                    y = wino_mm(ypool, w_a[1], 0, dxs)
                    lb = act_to(apool, y, 1, "aL")
                    for h in range(_SR):
                        v = lb[:, h, :][:, ::-1]
                        nc.vector.tensor_tensor_scan(v, v, v, 0.0,
                                                     op0=Alu.max, op1=Alu.bypass)
                    nc.sync.dma_start(lp_d.ap()[:, _SR * s:_SR * (s + 1), :], lb[:])

                    # conv B: forward cummax along H (ladder + carry)
                    y = wino_mm(ypool, w_a[2], 0, dxs)
                    bb = act_to(apool, y, 2, "aB")
                    nc.vector.tensor_tensor(bb[:, 1:8], bb[:, 1:8], bb[:, 0:7],
                                            Alu.max)
                    nc.vector.tensor_tensor(bb[:, 2:8], bb[:, 2:8], bb[:, 0:6],
                                            Alu.max)
                    nc.vector.tensor_tensor(bb[:, 4:8], bb[:, 4:8], bb[:, 0:4],
                                            Alu.max)
                    nc.vector.tensor_tensor(bb[:], bb[:],
                                            carry_b[:].broadcast_to(
                                                [128, _SR, 128]), Alu.max)
                    if s != S - 1:
                        nxt = cpool.tile([128, 1, 128], dt.bfloat16, tag="cryB")
                        nc.vector.tensor_copy(nxt[:], bb[:, 7:8])
                        carry_b = nxt

                    # conv R: forward cummax along W, sum with bp -> sum_br
                    y = wino_mm(ypool, w_a[3], 0, dxs)
                    rb = act_to(apool, y, 3, "aR")
                    for h in range(_SR):
                        v = rb[:, h, :]
                        nc.vector.tensor_tensor_scan(v, v, v, 0.0,
                                                     op0=Alu.max, op1=Alu.bypass)
                    sw = apool.tile([128, _SR, 130], dt.bfloat16, tag="swB")
                    nc.gpsimd.memset(sw[:, :, 0:1], 0.0)
                    nc.gpsimd.memset(sw[:, :, 129:130], 0.0)
                    nc.vector.tensor_tensor(sw[:, :, 1:129], bb[:], rb[:], Alu.add)
                    nc.sync.dma_start(
                        sum_d.ap()[1][:, 1 + _SR * s:1 + _SR * (s + 1), :], sw[:])

            # ---- pass A2: reverse-H pool over t, sum with lp -> sum_tl -----
            with tc.tile_pool(name="apA2", bufs=2) as a2pool:
                carry_t = cpool.tile([128, 1, 128], dt.bfloat16, tag="cryT")
                nc.vector.memset(carry_t[:], 0.0)
                for s in reversed(range(S)):
                    tb = a2pool.tile([128, _SR, 128], dt.bfloat16, tag="tA2")
                    lb = a2pool.tile([128, _SR, 128], dt.bfloat16, tag="lA2")
                    nc.gpsimd.dma_start(tb[:],
                                        t_d.ap()[:, _SR * s:_SR * (s + 1), :])
                    nc.gpsimd.dma_start(lb[:],
                                        lp_d.ap()[:, _SR * s:_SR * (s + 1), :])
                    nc.vector.tensor_tensor(tb[:, 0:7], tb[:, 0:7], tb[:, 1:8],
                                            Alu.max)
                    nc.vector.tensor_tensor(tb[:, 0:6], tb[:, 0:6], tb[:, 2:8],
                                            Alu.max)
                    nc.vector.tensor_tensor(tb[:, 0:4], tb[:, 0:4], tb[:, 4:8],
                                            Alu.max)
                    nc.vector.tensor_tensor(tb[:], tb[:],
                                            carry_t[:].broadcast_to(
                                                [128, _SR, 128]), Alu.max)
                    if s != 0:
                        nxt = cpool.tile([128, 1, 128], dt.bfloat16, tag="cryT")
                        nc.vector.tensor_copy(nxt[:], tb[:, 0:1])
                        carry_t = nxt
                    sw = a2pool.tile([128, _SR, 130], dt.bfloat16, tag="swT")
                    nc.gpsimd.memset(sw[:, :, 0:1], 0.0)
                    nc.gpsimd.memset(sw[:, :, 129:130], 0.0)
                    nc.vector.tensor_tensor(sw[:, :, 1:129], tb[:], lb[:], Alu.add)
                    nc.gpsimd.dma_start(
                        sum_d.ap()[0][:, 1 + _SR * s:1 + _SR * (s + 1), :], sw[:])

            # ---- stage C: tl = relu(wino3x3(sum) + conv1x1(x)) -------------
            with contextlib.ExitStack() as cctx:
                wpool = cctx.enter_context(tc.tile_pool(name="wpC", bufs=2))
                xpool = cctx.enter_context(tc.tile_pool(name="xpC", bufs=2))
                dpool = cctx.enter_context(tc.tile_pool(name="dpC", bufs=2))
                ypool = cctx.enter_context(tc.tile_pool(name="ypC", bufs=3))
                apool = cctx.enter_context(tc.tile_pool(name="apC", bufs=2))
                for bi in (1, 0):
                    w_c = load_w(wpool, wc_d.ap()[bi], 28, "wc")
                    for s in range(S):
                        ss = xpool.tile([128, NR, 130], dt.bfloat16, tag="ss")
                        nc.sync.dma_start(
                            ss[:], sum_d.ap()[bi][:, _SR * s:_SR * s + NR, :])
                        ds = wtransform(dpool, ss, "dsC")
                        xi = []
                        for ci in range(2):
                            t = xpool.tile([128, _SR, 128], dt.float32r,
                                           tag=f"xi{ci}")
                            nc.scalar.dma_start(
                                t[:],
                                xpad.ap()[ci][:, 1 + _SR * s:1 + _SR * (s + 1),
                                              1:129].bitcast(dt.float32r))
                            nc.vector.tensor_copy(t[:], t[:].bitcast(dt.float32))
                            xi.append(t)
                        for co in range(2):
                            c1 = []
                            for half in range(2):
                                ps = pspool.tile([128, 512], dt.float32, tag="ps")
                                for ci in range(2):
                                    nc.tensor.matmul(
                                        ps[:], w_c[:, 24 + co * 2 + ci],
                                        xi[ci][:, half * 4:half * 4 + 4, :],
                                        start=(ci == 0), stop=(ci == 1))
                                c1.append(ps[:].rearrange(
                                    "p (r g t) -> p r g t", g=64, t=2))
                            y = wino_mm(ypool, w_c, co * 12, [ds])
                            nc.vector.tensor_tensor(y[:, 0:4], y[:, 0:4], c1[0],
                                                    Alu.add)
                            nc.vector.tensor_tensor(y[:, 4:8], y[:, 4:8], c1[1],
                                                    Alu.add)
                            cw = apool.tile([128, _SR, 130], dt.bfloat16,
                                            tag="cw")
                            nc.gpsimd.memset(cw[:, :, 0:1], 0.0)
                            nc.gpsimd.memset(cw[:, :, 129:130], 0.0)
                            nc.scalar.activation(
                                cw[:, :, 1:129], y[:],
                                Act.Relu,
                                bias=bt[:, 4 + bi * 2 + co:5 + bi * 2 + co],
                                scale=1.0)
                            nc.sync.dma_start(
                                tlb_d.ap()[bi, co][:,
                                                   1 + _SR * s:1 + _SR * (s + 1),
                                                   :], cw[:])

            # ---- stage D: out = relu(wino3x3(tl)) --------------------------
            with contextlib.ExitStack() as dctx:
                wpool = dctx.enter_context(tc.tile_pool(name="wpD", bufs=2))
                xpool = dctx.enter_context(tc.tile_pool(name="xpD", bufs=2))
                dpool = dctx.enter_context(tc.tile_pool(name="dpD", bufs=2))
                ypool = dctx.enter_context(tc.tile_pool(name="ypD", bufs=3))
                apool = dctx.enter_context(tc.tile_pool(name="apD", bufs=2))
                for bi in (1, 0):
                    w_d = load_w(
                        wpool,
                        wd_d.ap()[bi].rearrange("p a b c -> p (a b) c"), 48, "wd")
                    out_d = out_tl if bi == 0 else out_br
                    for s in range(S):
                        dts = []
                        for ci in range(2):
                            t = xpool.tile([128, NR, 130], dt.bfloat16,
                                           tag=f"tl{ci}")
                            nc.sync.dma_start(
                                t[:],
                                tlb_d.ap()[bi, ci][:, _SR * s:_SR * s + NR, :])
                            dts.append(wtransform(dpool, t, f"dtD{ci}"))
                        for co in range(2):
                            y = wino_mm(ypool, w_d, co * 24, dts)
                            ot = act_to(apool, y, 8 + bi * 2 + co, "oD",
                                        dtype=dt.float32)
                            nc.sync.dma_start(
                                out_d.ap()[co * 128:(co + 1) * 128,
                                           _SR * s:_SR * (s + 1), :], ot[:])

    nc.compile()
    return nc


_NC_CACHE = {}


def _get_nc(H):
    if H not in _NC_CACHE:
        _NC_CACHE[H] = _build(H)
    return _NC_CACHE[H]


def kernel(**inputs):
    from concourse import bass_utils

    x = np.asarray(inputs["x"], np.float32)
    B, C, H, W = x.shape
    assert (C, W) == (256, 128) and H % _SR == 0

    shared = _prep_host(inputs)
    nc = _get_nc(H)

    in_maps = []
    for b in range(B):
        m = dict(shared)
        m["xpad"] = _pad_x_sample(x[b], H)
        in_maps.append(m)

    import os
    trace = bool(int(os.environ.get("KERNEL_TRACE", "0")))
    res = bass_utils.run_bass_kernel_spmd(
        nc, in_maps, core_ids=list(range(B)), trace=trace)
    kernel.last_result = res

    otl = np.stack([res.results[b]["out_tl"].reshape(256, H, 128)
                    for b in range(B)])
    obr = np.stack([res.results[b]["out_br"].reshape(256, H, 128)
                    for b in range(B)])
    return otl, obr
